# revision 5
# baseline (speedup 1.0000x reference)
"""Multi-head attention on 8 TRN2 NeuronCores (data/head-parallel).

Problem: B=4 H=16 S=2048 D=64 fp32 attention, out = softmax(Q K^T / sqrt(D)) V.
B*H = 64 (batch, head) pairs are sharded 8-per-core; each core runs the same
NEFF over its own 8 heads, no collectives.

Per-head dataflow on a core (all matmuls bf16 in / fp32 PSUM out):
  - DMA Q,K,V f32 -> SBUF; GpSimd converts to bf16; Q and K are packed
    side-by-side into [128, 128] tiles and DMA-xbar-transposed so SBUF holds
    Q^T and K^T with the head dim (64) on partitions.
  - S^T[k, q] = K^T.T @ Q^T on PE  (contraction over d=64).
  - E^T = exp(S^T / sqrt(D)) on ACT (PSUM -> SBUF bf16); the 1/sqrt(D) scale
    rides the activation's free affine input scale.
  - out'^T[d', q] = sum_k V'[k, d'].T @ E^T[k, q] on PE, where V' has a ones
    column appended (d'=65) so row 64 accumulates the softmax denominators.
  - PE transposes out'^T back to [q, 65] tiles; DVE computes reciprocal of
    the denominator column and scales; DMA out f32.
"""

import math
from contextlib import ExitStack

import numpy as np

import concourse.bass as bass
import concourse.bacc as bacc
import concourse.tile as tile
import concourse.mybir as mybir
from concourse.bass_utils import run_bass_kernel_spmd
from concourse.masks import make_identity

B, H, S, D = 4, 16, 2048, 64
N_CORES = 8
HPC = B * H // N_CORES     # heads per core
ST = S // 128              # 16 s-tiles of 128
QCHUNK = 1024              # q processed in chunks (PSUM budget)
NQ = S // QCHUNK
MMN = 512                  # moving free dim per matmul (one PSUM bank)
DT = mybir.dt

_BUILT = {}


def _build_head(nc, tc, ctx, pools, id65, scale, q_d, k_d, v_d, o_d, h):
    (stage, epool, spool, outp, ps_st, ps_ot, ps_tt) = pools

    # ---- load + convert + transpose --------------------------------------
    qf = stage.tile([128, ST, D], DT.float32, tag="qf")
    kf = stage.tile([128, ST, D], DT.float32, tag="kf")
    vf = stage.tile([128, ST, D], DT.float32, tag="vf")
    # s = t*128 + p  ->  partition p, free (t, d)
    q_v = q_d[h].rearrange("(t p) d -> p t d", p=128)
    k_v = k_d[h].rearrange("(t p) d -> p t d", p=128)
    v_v = v_d[h].rearrange("(t p) d -> p t d", p=128)
    for j in range(4):
        sl = slice(4 * j, 4 * j + 4)
        nc.gpsimd.dma_start(out=qf[:, sl, :], in_=q_v[:, sl, :])
        nc.gpsimd.dma_start(out=kf[:, sl, :], in_=k_v[:, sl, :])
        nc.gpsimd.dma_start(out=vf[:, sl, :], in_=v_v[:, sl, :])

    qkb = stage.tile([128, ST, 128], DT.bfloat16, tag="qkb")
    nc.gpsimd.tensor_copy(out=qkb[:, :, 0:D], in_=qf)
    nc.gpsimd.tensor_copy(out=qkb[:, :, D:128], in_=kf)
    vb = stage.tile([128, ST, D + 1], DT.bfloat16, tag="vb")
    nc.gpsimd.tensor_copy(out=vb[:, :, 0:D], in_=vf)
    nc.gpsimd.memset(vb[:, :, D : D + 1], 1.0)

    # xbar transpose: [128 s, 128(dQ|dK)] -> [128(dQ|dK), 128 s]
    qkt = stage.tile([128, S], DT.bfloat16, tag="qkt")
    for t in range(ST):
        nc.sync.dma_start(
            out=qkt[:, t * 128 : (t + 1) * 128], in_=qkb[:, t, :], transpose=True
        )
    qT = qkt[0:D, :]         # [64, 2048]
    # matmul needs both operands at base partition 0; move K^T down.
    kt = stage.tile([D, S], DT.bfloat16, tag="kt")
    for j in range(2):
        half = slice(j * (S // 2), (j + 1) * (S // 2))
        nc.sync.dma_start(out=kt[:, half], in_=qkt[D:128, half])
    kT = kt

    # ---- attention per q-chunk -------------------------------------------
    for c in range(NQ):
        q0 = c * QCHUNK
        ets = []
        for t in range(ST):
            st = ps_st.tile([128, QCHUNK], DT.float32, tag="st")
            for n in range(QCHUNK // MMN):
                nc.tensor.matmul(
                    st[:, n * MMN : (n + 1) * MMN],
                    lhsT=kT[:, t * 128 : (t + 1) * 128],
                    rhs=qT[:, q0 + n * MMN : q0 + (n + 1) * MMN],
                    start=True,
                    stop=True,
                )
            et = epool.tile([128, QCHUNK], DT.bfloat16, tag=f"et{t}")
            nc.scalar.activation(
                out=et, in_=st, func=mybir.ActivationFunctionType.Exp, scale=scale
            )
            ets.append(et)

        ot = ps_ot.tile([D + 1, QCHUNK], DT.float32, tag="ot")
        for t in range(ST):
            for n in range(QCHUNK // MMN):
                nc.tensor.matmul(
                    ot[:, n * MMN : (n + 1) * MMN],
                    lhsT=vb[:, t, :],
                    rhs=ets[t][:, n * MMN : (n + 1) * MMN],
                    start=(t == 0),
                    stop=(t == ST - 1),
                )

        # ---- normalize: transpose back, scale by 1/denominator ----------
        ots = spool.tile([D + 1, QCHUNK], DT.float32, tag="ots")
        nc.vector.tensor_copy(out=ots, in_=ot)
        outst = outp.tile([128, QCHUNK // 128, D], DT.float32, tag="outst")
        nquad = QCHUNK // (4 * 128)
        for g in range(nquad):
            tt = ps_tt.tile([128, 4 * (D + 1)], DT.float32, tag="tt")
            for j in range(4):
                r = 4 * g + j
                nc.tensor.transpose(
                    tt[:, j * (D + 1) : (j + 1) * (D + 1)],
                    ots[:, r * 128 : (r + 1) * 128],
                    id65,
                )
            ttv = tt.rearrange("p (j x) -> p j x", j=4)
            rec = spool.tile([128, 4], DT.float32, tag="rec")
            nc.vector.reciprocal(out=rec, in_=ttv[:, :, D])
            for j in range(4):
                nc.vector.tensor_scalar(
                    outst[:, 4 * g + j, :],
                    ttv[:, j, 0:D],
                    rec[:, j : j + 1],
                    None,
                    mybir.AluOpType.mult,
                )
        o_v = o_d[h, q0 : q0 + QCHUNK, :].rearrange("(r p) d -> p r d", p=128)
        nc.gpsimd.dma_start(out=o_v, in_=outst)


def build_graph(scale: float, heads: int = HPC):
    nc = bacc.Bacc("TRN2", target_bir_lowering=False, debug=False,
                   num_devices=N_CORES)
    q_d = nc.dram_tensor("Q", [heads, S, D], DT.float32, kind="ExternalInput").ap()
    k_d = nc.dram_tensor("K", [heads, S, D], DT.float32, kind="ExternalInput").ap()
    v_d = nc.dram_tensor("V", [heads, S, D], DT.float32, kind="ExternalInput").ap()
    o_d = nc.dram_tensor("out", [heads, S, D], DT.float32, kind="ExternalOutput").ap()

    with tile.TileContext(nc) as tc, ExitStack() as ctx:
        const = ctx.enter_context(tc.tile_pool(name="const", bufs=1))
        stage = ctx.enter_context(tc.tile_pool(name="stage", bufs=2))
        epool = ctx.enter_context(tc.tile_pool(name="epool", bufs=2))
        spool = ctx.enter_context(tc.tile_pool(name="spool", bufs=2))
        outp = ctx.enter_context(tc.tile_pool(name="outp", bufs=2))
        ps_st = ctx.enter_context(tc.tile_pool(name="ps_st", bufs=2, space="PSUM"))
        ps_ot = ctx.enter_context(tc.tile_pool(name="ps_ot", bufs=1, space="PSUM"))
        ps_tt = ctx.enter_context(tc.tile_pool(name="ps_tt", bufs=2, space="PSUM"))

        id65 = const.tile([D + 1, D + 1], DT.float32)
        make_identity(nc, id65)

        pools = (stage, epool, spool, outp, ps_st, ps_ot, ps_tt)
        for h in range(heads):
            _build_head(nc, tc, ctx, pools, id65, scale, q_d, k_d, v_d, o_d, h)

    nc.compile()
    return nc


def _get_nc(scale: float):
    key = round(float(scale), 9)
    if key not in _BUILT:
        _BUILT[key] = build_graph(float(scale))
    return _BUILT[key]


def shard_inputs(Q, K, V):
    qs = np.ascontiguousarray(np.asarray(Q, dtype=np.float32).reshape(B * H, S, D))
    ks = np.ascontiguousarray(np.asarray(K, dtype=np.float32).reshape(B * H, S, D))
    vs = np.ascontiguousarray(np.asarray(V, dtype=np.float32).reshape(B * H, S, D))
    in_maps = []
    for c in range(N_CORES):
        sl = slice(c * HPC, (c + 1) * HPC)
        in_maps.append({
            "Q": np.ascontiguousarray(qs[sl]),
            "K": np.ascontiguousarray(ks[sl]),
            "V": np.ascontiguousarray(vs[sl]),
        })
    return in_maps


def kernel(Q, K, V, d_k, **run_kwargs):
    scale = 1.0 / math.sqrt(float(d_k))
    nc = _get_nc(scale)
    in_maps = shard_inputs(Q, K, V)
    res = run_bass_kernel_spmd(nc, in_maps, core_ids=list(range(N_CORES)),
                               **run_kwargs)
    out = np.concatenate([r["out"] for r in res.results], axis=0)
    out = out.reshape(B, H, S, D).astype(np.float32)
    kernel.last_results = res
    return out
